# revision 25
# baseline (speedup 1.0000x reference)
"""Multi-head attention (B=8, N=1024, C=768, H=12) on 8 TRN2 NeuronCores.

Sharding: data-parallel over batch — core i computes batch element i fully.
Weights / bias tables are replicated. No collectives.

Key ideas (all matmuls bf16, f32 PSUM accumulation):
  * Key compaction: the key_padding_mask invalidates ~half the keys. The host
    gathers the valid key rows of x into a compacted, zero-padded key-side
    input (NVpad rows), so S^T / exp / PV / V-projection shrink by ~NVpad/N.
    Padded rows carry zero V and a zero mask column, so they contribute
    nothing to either the numerator or the softmax denominator.
  * Flipped attention S^T[j,i] (compacted keys on partitions):
    P = exp(S^T) * ebias, where ebias is the host-gathered exp() of the
    relative-position bias for each (compacted key, query) pair
    (exp(S+B) = exp(S)exp(B)); no row-max subtraction needed (|S| <= ~10).
  * O^T_unnorm[d,i] and the softmax denominator come from ONE matmul per
    (jt, i-chunk): lhsT = [V | mask] puts the masked softmax sum in PSUM
    row 64.
  * Normalization is lazy (recip = exp(-ln(d)) on the Scalar engine,
    broadcast via a DRAM bounce) and grouped so it overlaps later compute.
  * Output projection consumes O^T directly as lhsT (c-major layout), adds
    b_proj, writes f32 [token, feature] tiles.
"""

import numpy as np
import ml_dtypes

DIM = 768
NUM_HEADS = 12
HD = 64
N_TOK = 1024
B = 8
SCALE = HD ** -0.5

_BUILD_CACHE = {}


def _build_nc(N=N_TOK, H=NUM_HEADS, NVT=None, mmdt_name="bfloat16"):
    import concourse.bass as bass
    import concourse.mybir as mybir
    import concourse.tile as tile
    from concourse import bacc

    # Pin every activation to the one table set containing both exp and ln,
    # so the Scalar engine never thrashes ACT_TABLE_LOADs between the softmax
    # exp stream and the ln/exp reciprocal. Other sets are emptied (indices
    # must stay aligned with act_info.json, so no reordering/filtering).
    if not getattr(bacc, "_act_tables_pinned", False):
        _orig_gat = bacc.get_activation_tables

        def _pinned_gat(arch):
            tabs = _orig_gat(arch)
            want = None
            for name, funcs in tabs.items():
                fn = {f.name.lower() for f in funcs}
                if "exp" in fn and "ln" in fn and "copy" in fn:
                    want = name
                    break
            if want is None:
                return tabs
            return {
                name: (funcs if name == want else set())
                for name, funcs in tabs.items()
            }

        bacc.get_activation_tables = _pinned_gat
        bacc._act_tables_pinned = True

    f32 = mybir.dt.float32
    mmdt = getattr(mybir.dt, mmdt_name)
    Exp = mybir.ActivationFunctionType.Exp
    Ln = mybir.ActivationFunctionType.Ln
    mult = mybir.AluOpType.mult
    add = mybir.AluOpType.add

    C = H * HD                      # 768
    NT = N // 128                   # query tiles
    if NVT is None:
        NVT = NT                    # compacted key tiles
    NV = NVT * 128
    KO = C // 128                   # contraction slots (== head pairs HP)
    HP = H // 2
    ichunks = [(i0, min(512, N - i0)) for i0 in range(0, N, 512)]
    kchunks = [(k0, min(512, NV - k0)) for k0 in range(0, NV, 512)]
    fchunks = [(f0, min(512, C - f0)) for f0 in range(0, C, 512)]

    nc = bacc.Bacc(None)
    xT_d = nc.declare_dram_parameter("xT", [C, N], mmdt, isOutput=False)
    xkT_d = nc.declare_dram_parameter("xkT", [C, NV], mmdt, isOutput=False)
    wqk_d = nc.declare_dram_parameter("wqk", [C, 2 * C], mmdt, isOutput=False)
    wv_d = nc.declare_dram_parameter("wv", [C, C], mmdt, isOutput=False)
    wp_d = nc.declare_dram_parameter("wp", [C, C], mmdt, isOutput=False)
    ebias_d = nc.declare_dram_parameter("ebias", [H, NV, N], mmdt, isOutput=False)
    mask_d = nc.declare_dram_parameter("maskc", [128, NVT], mmdt, isOutput=False)
    bp_d = nc.declare_dram_parameter("bproj", [C], f32, isOutput=False)
    out_d = nc.declare_dram_parameter("out", [N, C], f32, isOutput=True)

    with tile.TileContext(nc) as tc:
        with (
            tc.tile_pool(name="singles", bufs=1) as singles,
            tc.tile_pool(name="dram", bufs=1, space="DRAM") as drampool,
        ):
            # ---- input loads, most-urgent first (xk/wv feed the V matmuls,
            # ---- which open the PE stream) ----
            KH = (KO + 1) // 2
            xkT_r = xkT_d.rearrange("(ko p) n -> p ko n", p=128)
            xka = singles.tile([128, KH, NV], mmdt)
            nc.sync.dma_start(xka[:], xkT_r[:, :KH])
            wv = singles.tile([128, KO, C], mmdt)
            nc.sync.dma_start(wv[:], wv_d.rearrange("(ko p) m -> p ko m", p=128))
            maskc = singles.tile([128, NVT], mmdt)
            nc.sync.dma_start(maskc[:], mask_d[:])
            xkb = None
            if KH < KO:
                xkb = singles.tile([128, KO - KH, NV], mmdt)
                nc.sync.dma_start(xkb[:], xkT_r[:, KH:])

            def xkTs(ko):
                return xka[:, ko] if ko < KH else xkb[:, ko - KH]

            xT = singles.tile([128, KO, N], mmdt)
            d_xT = nc.sync.dma_start(
                xT[:], xT_d.rearrange("(ko p) n -> p ko n", p=128)
            )
            wqk = singles.tile([128, KO, 2 * C], mmdt)
            d_wqk = nc.sync.dma_start(
                wqk[:], wqk_d.rearrange("(ko p) m -> p ko m", p=128)
            )
            wp = singles.tile([128, KO, C], mmdt)
            d_wp = nc.sync.dma_start(
                wp[:], wp_d.rearrange("(ko p) m -> p ko m", p=128)
            )
            bp = singles.tile([128, C], f32)
            d_bp = nc.sync.dma_start(
                bp[:],
                bass.AP(tensor=bp_d, offset=0, ap=[[0, 128], [1, C]]),
            )

            qt = singles.tile([128, HP, N], mmdt)
            kt = singles.tile([128, HP, NV], mmdt)
            vsb = singles.tile([128, NVT, H, HD + 1], mmdt)
            ou = singles.tile([128, HP, N], mmdt)      # unnormalized O^T (packed)
            rb = singles.tile([128, HP, N], mmdt)      # broadcast recips (packed)
            den = singles.tile([128, N], f32)          # denominators (mapped rows)
            rden = singles.tile([128, N], f32)
            rdb = singles.tile([128, N], mmdt)         # bf16 recip denominators
            rscratch = drampool.tile([H, N], mmdt)

            with (
                tc.tile_pool(name="qkv_psum", bufs=2, space="PSUM") as qp,
                tc.tile_pool(name="eb_pool", bufs=2) as eb_pool,
                tc.tile_pool(name="st_psum", bufs=2, space="PSUM") as st_psum,
                tc.tile_pool(name="pv_psum", bufs=len(ichunks), space="PSUM") as pv_psum,
                tc.tile_pool(name="e_pool", bufs=4) as e_pool,
                tc.tile_pool(name="p_pool", bufs=5) as p_pool,
                tc.tile_pool(name="drow_pool", bufs=3) as drow_pool,
            ):
                if HP >= 5:
                    # (first head, den partition start, n heads) by trigger pair
                    NORM_GROUPS = {
                        2: (0, 0, 6),
                        HP - 2: (6, 32, 2 * HP - 8),
                        HP - 1: (2 * HP - 2, 64, 2),
                    }
                else:
                    NORM_GROUPS = {HP - 1: (0, 0, H)}
                DEN_ROW = {}
                for _g0, _r0, _ng in NORM_GROUPS.values():
                    for _h in range(_g0, _g0 + _ng):
                        DEN_ROW[_h] = _r0 + _h - _g0

                def _normalize_group(g0, r0, ng):
                    nc.scalar.activation(
                        rden[r0 : r0 + ng], den[r0 : r0 + ng], Ln
                    )
                    nc.scalar.activation(
                        rdb[r0 : r0 + ng], rden[r0 : r0 + ng], Exp, scale=-1.0
                    )
                    nc.sync.dma_start(rscratch[g0 : g0 + ng], rdb[r0 : r0 + ng])
                    for h in range(g0, g0 + ng):
                        ho = 64 * (h % 2)
                        nc.sync.dma_start(
                            rb[ho : ho + 64, h // 2, :],
                            bass.AP(
                                tensor=rscratch.tensor,
                                offset=rscratch[h, 0].offset,
                                ap=[[0, 64], [1, N]],
                            ),
                        )
                    for sl in range(g0 // 2, (g0 + ng) // 2):
                        nc.vector.tensor_tensor(
                            ou[:, sl, :], ou[:, sl, :], rb[:, sl, :], mult
                        )

                # ---- V projection over compacted keys, masked, into [V|m] ----
                from concourse.tile import add_dep_helper
                _dma_anchor = None
                for jt in range(NVT):
                    for f0, fl in fchunks:
                        ps = qp.tile([128, 512], f32, tag="ps")
                        for ko in range(KO):
                            mm = nc.tensor.matmul(
                                ps[:, :fl],
                                lhsT=xkTs(ko)[:, 128 * jt : 128 * jt + 128],
                                rhs=wv[:, ko, f0 : f0 + fl],
                                start=(ko == 0),
                                stop=(ko == KO - 1),
                            )
                        if _dma_anchor is None:
                            _dma_anchor = mm.ins
                            for _d in (d_xT, d_wqk, d_wp, d_bp):
                                add_dep_helper(
                                    _d.ins,
                                    _dma_anchor,
                                    sync=True,
                                    reason="delay bulk loads behind urgent ones",
                                )
                        h0, nh = f0 // HD, fl // HD
                        nc.vector.tensor_tensor(
                            vsb[:, jt, h0 : h0 + nh, 0:HD],
                            ps[:, :fl].rearrange("p (h d) -> p h d", d=HD),
                            maskc[:, jt : jt + 1, None].to_broadcast([128, nh, HD]),
                            mult,
                        )
                    # mask column (the "ones" column that accumulates denom)
                    nc.vector.tensor_scalar_mul(
                        vsb[:, jt, :, HD : HD + 1],
                        maskc[:, jt : jt + 1, None].to_broadcast([128, H, 1]),
                        1.0,
                    )

                for pair in range(HP):
                    # Q^T (from x) / K^T (from compacted xk) for this pair.
                    # Chunk-interleaved emission (Q0, K0, Q1, K1) so the first
                    # S^T matmul's operands (Q chunk 0 + K chunk 0) cast out
                    # while the later chunks' matmuls still run on the PE.
                    qk_groups = []
                    for ci in range(max(len(ichunks), len(kchunks))):
                        if ci < len(ichunks):
                            qk_groups.append((pair, qt, ichunks[ci]))
                        if ci < len(kchunks):
                            qk_groups.append((HP + pair, kt, kchunks[ci]))
                    for mt, dst, (i0, il) in qk_groups:
                        ps = qp.tile([128, 512], f32, tag="ps")
                        for ko in range(KO):
                            rhs = (
                                xT[:, ko, i0 : i0 + il]
                                if mt < HP
                                else xkTs(ko)[:, i0 : i0 + il]
                            )
                            nc.tensor.matmul(
                                ps[:, :il],
                                lhsT=wqk[:, ko, 128 * mt : 128 * mt + 128],
                                rhs=rhs,
                                start=(ko == 0),
                                stop=(ko == KO - 1),
                            )
                        nc.vector.tensor_copy(
                            dst[:, pair, i0 : i0 + il], ps[:, :il]
                        )
                    # attention for the pair's two heads
                    for h in (2 * pair, 2 * pair + 1):
                        hp, ho = pair, 64 * (h % 2)
                        eb = eb_pool.tile([128, NVT, N], mmdt, tag="eb")
                        d_eb = nc.sync.dma_start(
                            eb[:], ebias_d[h].rearrange("(jt p) n -> p jt n", p=128)
                        )
                        if h < 2:
                            add_dep_helper(
                                d_eb.ins,
                                _dma_anchor,
                                sync=True,
                                reason="delay early ebias behind urgent loads",
                            )
                        pvs = [
                            pv_psum.tile([128, 512], f32, tag="pv", name=f"pv_{h}_{ic}")
                            for ic in range(len(ichunks))
                        ]
                        ptiles = {}
                        for jt in range(NVT + 2):
                            if jt < NVT:
                                st = st_psum.tile([128, N], f32, tag="st")
                                for i0, il in ichunks:
                                    nc.tensor.matmul(
                                        st[:, i0 : i0 + il],
                                        lhsT=kt[ho : ho + 64, hp, 128 * jt : 128 * jt + 128],
                                        rhs=qt[ho : ho + 64, hp, i0 : i0 + il],
                                        start=True,
                                        stop=True,
                                    )
                                e = e_pool.tile([128, N], mmdt, tag="e")
                                nc.scalar.activation(e[:], st[:], Exp)
                                p = p_pool.tile([128, N], mmdt, tag="p")
                                nc.vector.tensor_tensor(
                                    p[:], e[:], eb[:, jt, :], mult
                                )
                                ptiles[jt] = p
                            if jt >= 2:
                                jd = jt - 2
                                pd = ptiles.pop(jd)
                                for ic, (i0, il) in enumerate(ichunks):
                                    nc.tensor.matmul(
                                        pvs[ic][: HD + 1, :il],
                                        lhsT=vsb[:, jd, h, :],
                                        rhs=pd[:, i0 : i0 + il],
                                        start=(jd == 0),
                                        stop=(jd == NVT - 1),
                                    )
                        for ic, (i0, il) in enumerate(ichunks):
                            nc.vector.tensor_copy(
                                ou[ho : ho + 64, hp, i0 : i0 + il],
                                pvs[ic][:HD, :il],
                            )
                            drow = drow_pool.tile(
                                [128, 512], f32, tag="drow", name=f"drow_{h}_{ic}"
                            )
                            nc.scalar.copy(
                                drow[64:65, :il], pvs[ic][HD : HD + 1, :il]
                            )
                            dr = DEN_ROW[h]
                            nc.sync.dma_start(
                                den[dr : dr + 1, i0 : i0 + il], drow[64:65, :il]
                            )
                    # normalize in groups (32-aligned partition starts for the
                    # recip ops); overlaps later pairs' compute
                    if pair in NORM_GROUPS:
                        _normalize_group(*NORM_GROUPS[pair])

            # ---------------- output projection ----------------
            # ko 0..KO-2 run eagerly (their O^T slots normalize early); each
            # group's final-ko matmul (the slot normalized in the very last
            # group) is lagged so the normalize tail hides under real work.
            with (
                tc.tile_pool(name="proj_psum", bufs=7, space="PSUM") as proj_psum,
                tc.tile_pool(name="o_pool", bufs=3) as o_pool,
            ):
                groups = [(it, f0, fl) for it in range(NT) for f0, fl in fchunks]
                LAG = min(5, len(groups) - 1) if KO > 1 else 0
                psums = {}
                ots = {}
                for g in range(len(groups) + LAG):
                    if g < len(groups):
                        it, f0, fl = groups[g]
                        ps = proj_psum.tile(
                            [128, 512], f32, tag="ps", name=f"pps_{g}"
                        )
                        for ko in range(KO - 1):
                            nc.tensor.matmul(
                                ps[:, :fl],
                                lhsT=ou[:, ko, 128 * it : 128 * it + 128],
                                rhs=wp[:, ko, f0 : f0 + fl],
                                start=(ko == 0),
                                stop=False,
                            )
                        psums[g] = ps
                    if g >= LAG:
                        it, f0, fl = groups[g - LAG]
                        ps = psums.pop(g - LAG)
                        nc.tensor.matmul(
                            ps[:, :fl],
                            lhsT=ou[:, KO - 1, 128 * it : 128 * it + 128],
                            rhs=wp[:, KO - 1, f0 : f0 + fl],
                            start=(KO == 1),
                            stop=True,
                        )
                        if it not in ots:
                            ots[it] = o_pool.tile(
                                [128, C], f32, tag="ot", name=f"ot_{it}"
                            )
                        nc.vector.tensor_tensor(
                            ots[it][:, f0 : f0 + fl],
                            ps[:, :fl],
                            bp[:, f0 : f0 + fl],
                            add,
                        )
                        if f0 + fl >= C:
                            nc.sync.dma_start(
                                out_d[128 * it : 128 * it + 128, :], ots.pop(it)[:]
                            )

    nc.finalize()
    return nc


def _host_pack(x, w_qkv, w_proj, b_proj, bias_table, key_padding_mask,
               N=N_TOK, H=NUM_HEADS, mmdt_name="bfloat16"):
    """Host-side layout: per-core input dicts (core i <- batch i).
    Returns (in_maps, NVT)."""
    np_mmdt = ml_dtypes.bfloat16 if mmdt_name == "bfloat16" else np.float32
    C = H * HD

    x = np.asarray(x, np.float32)
    mask = np.asarray(key_padding_mask).astype(bool)
    Bb = x.shape[0]

    valid = [np.where(mask[b])[0] for b in range(Bb)]
    nv_max = max(1, max(len(v) for v in valid))
    NVT = (nv_max + 127) // 128
    NV = NVT * 128

    w_qkv = np.asarray(w_qkv, np.float32)
    wqk = np.ascontiguousarray(w_qkv[: 2 * C].T).astype(np.float32)
    wqk[:, :C] *= SCALE                       # fold softmax scale into W_q
    wqk = wqk.astype(np_mmdt)
    wv = np.ascontiguousarray(w_qkv[2 * C :].T).astype(np_mmdt)
    wp = np.ascontiguousarray(np.asarray(w_proj, np.float32).T).astype(np_mmdt)
    bp = np.asarray(b_proj, np.float32)

    etab = np.exp(np.asarray(bias_table, np.float32))   # [2N-1, H]
    iota = np.arange(N)

    in_maps = []
    for b in range(Bb):
        v = valid[b]
        nv = len(v)
        xT = np.ascontiguousarray(x[b].T).astype(np_mmdt)
        xk = np.zeros((NV, C), np.float32)
        xk[:nv] = x[b][v]
        xkT = np.ascontiguousarray(xk.T).astype(np_mmdt)
        mc = np.zeros(NV, np.float32)
        mc[:nv] = 1.0
        maskc = np.ascontiguousarray(mc.reshape(NVT, 128).T).astype(np_mmdt)
        # ebias[h, r, i] = exp(bias_table[v[r] - i + N - 1, h])
        idx = np.zeros((NV, N), np.int32)
        idx[:nv] = v[:, None] - iota[None, :] + N - 1
        eb = etab[idx, :]                     # [NV, N, H]
        eb[nv:] = 0.0
        ebias = np.ascontiguousarray(eb.transpose(2, 0, 1)).astype(np_mmdt)
        in_maps.append({
            "xT": xT, "xkT": xkT, "wqk": wqk, "wv": wv, "wp": wp,
            "ebias": ebias, "maskc": maskc, "bproj": bp,
        })
    return in_maps, NVT


def _run(x, w_qkv, w_proj, b_proj, bias_table, key_padding_mask, trace=False):
    from concourse.bass_utils import run_bass_kernel_spmd

    in_maps, NVT = _host_pack(
        x, w_qkv, w_proj, b_proj, bias_table, key_padding_mask
    )
    key = ("full", N_TOK, NUM_HEADS, NVT)
    if key not in _BUILD_CACHE:
        _BUILD_CACHE[key] = _build_nc(NVT=NVT)
    nc = _BUILD_CACHE[key]
    res = run_bass_kernel_spmd(nc, in_maps, core_ids=list(range(B)), trace=trace)
    out = np.stack([np.asarray(res.results[i]["out"]) for i in range(B)])
    return out.astype(np.float32), res


def kernel(x, w_qkv, w_proj, b_proj, bias_table, key_padding_mask):
    out, _ = _run(x, w_qkv, w_proj, b_proj, bias_table, key_padding_mask)
    return out


# revision 26
# speedup vs baseline: 1.0089x; 1.0089x over previous
"""Multi-head attention (B=8, N=1024, C=768, H=12) on 8 TRN2 NeuronCores.

Sharding: data-parallel over batch — core i computes batch element i fully.
Weights / bias tables are replicated. No collectives.

Key ideas (all matmuls bf16, f32 PSUM accumulation):
  * Key compaction: the key_padding_mask invalidates ~half the keys. The host
    gathers the valid key rows of x into a compacted, zero-padded key-side
    input (NVpad rows), so S^T / exp / PV / V-projection shrink by ~NVpad/N.
    Padded rows carry zero V and a zero mask column, so they contribute
    nothing to either the numerator or the softmax denominator.
  * Flipped attention S^T[j,i] (compacted keys on partitions):
    P = exp(S^T) * ebias, where ebias is the host-gathered exp() of the
    relative-position bias for each (compacted key, query) pair
    (exp(S+B) = exp(S)exp(B)); no row-max subtraction needed (|S| <= ~10).
  * O^T_unnorm[d,i] and the softmax denominator come from ONE matmul per
    (jt, i-chunk): lhsT = [V | mask] puts the masked softmax sum in PSUM
    row 64.
  * Normalization is lazy (recip = exp(-ln(d)) on the Scalar engine,
    broadcast via a DRAM bounce) and grouped so it overlaps later compute.
  * Output projection consumes O^T directly as lhsT (c-major layout), adds
    b_proj, writes f32 [token, feature] tiles.
"""

import numpy as np
import ml_dtypes

DIM = 768
NUM_HEADS = 12
HD = 64
N_TOK = 1024
B = 8
SCALE = HD ** -0.5

_BUILD_CACHE = {}


def _build_nc(N=N_TOK, H=NUM_HEADS, NVT=None, mmdt_name="bfloat16"):
    import concourse.bass as bass
    import concourse.mybir as mybir
    import concourse.tile as tile
    from concourse import bacc

    # Pin every activation to the one table set containing both exp and ln,
    # so the Scalar engine never thrashes ACT_TABLE_LOADs between the softmax
    # exp stream and the ln/exp reciprocal. Other sets are emptied (indices
    # must stay aligned with act_info.json, so no reordering/filtering).
    if not getattr(bacc, "_act_tables_pinned", False):
        _orig_gat = bacc.get_activation_tables

        def _pinned_gat(arch):
            tabs = _orig_gat(arch)
            want = None
            for name, funcs in tabs.items():
                fn = {f.name.lower() for f in funcs}
                if "exp" in fn and "ln" in fn and "copy" in fn:
                    want = name
                    break
            if want is None:
                return tabs
            return {
                name: (funcs if name == want else set())
                for name, funcs in tabs.items()
            }

        bacc.get_activation_tables = _pinned_gat
        bacc._act_tables_pinned = True

    f32 = mybir.dt.float32
    mmdt = getattr(mybir.dt, mmdt_name)
    Exp = mybir.ActivationFunctionType.Exp
    Ln = mybir.ActivationFunctionType.Ln
    mult = mybir.AluOpType.mult
    add = mybir.AluOpType.add

    C = H * HD                      # 768
    NT = N // 128                   # query tiles
    if NVT is None:
        NVT = NT                    # compacted key tiles
    NV = NVT * 128
    KO = C // 128                   # contraction slots (== head pairs HP)
    HP = H // 2
    ichunks = [(i0, min(512, N - i0)) for i0 in range(0, N, 512)]
    kchunks = [(k0, min(512, NV - k0)) for k0 in range(0, NV, 512)]
    fchunks = [(f0, min(512, C - f0)) for f0 in range(0, C, 512)]

    nc = bacc.Bacc(None)
    xT_d = nc.declare_dram_parameter("xT", [C, N], mmdt, isOutput=False)
    xkT_d = nc.declare_dram_parameter("xkT", [C, NV], mmdt, isOutput=False)
    wqk_d = nc.declare_dram_parameter("wqk", [C, 2 * C], mmdt, isOutput=False)
    wv_d = nc.declare_dram_parameter("wv", [C, C], mmdt, isOutput=False)
    wp_d = nc.declare_dram_parameter("wp", [C, C], mmdt, isOutput=False)
    ebias_d = nc.declare_dram_parameter("ebias", [H, NV, N], mmdt, isOutput=False)
    mask_d = nc.declare_dram_parameter("maskc", [128, NVT], mmdt, isOutput=False)
    bp_d = nc.declare_dram_parameter("bproj", [C], f32, isOutput=False)
    out_d = nc.declare_dram_parameter("out", [N, C], f32, isOutput=True)

    with tile.TileContext(nc) as tc:
        with (
            tc.tile_pool(name="singles", bufs=1) as singles,
            tc.tile_pool(name="dram", bufs=1, space="DRAM") as drampool,
        ):
            # ---- input loads, most-urgent first (xk/wv feed the V matmuls,
            # ---- which open the PE stream) ----
            KH = (KO + 1) // 2
            xkT_r = xkT_d.rearrange("(ko p) n -> p ko n", p=128)
            xka = singles.tile([128, KH, NV], mmdt)
            nc.sync.dma_start(xka[:], xkT_r[:, :KH])
            wv = singles.tile([128, KO, C], mmdt)
            nc.sync.dma_start(wv[:], wv_d.rearrange("(ko p) m -> p ko m", p=128))
            maskc = singles.tile([128, NVT], mmdt)
            nc.sync.dma_start(maskc[:], mask_d[:])
            xkb = None
            if KH < KO:
                xkb = singles.tile([128, KO - KH, NV], mmdt)
                nc.sync.dma_start(xkb[:], xkT_r[:, KH:])

            def xkTs(ko):
                return xka[:, ko] if ko < KH else xkb[:, ko - KH]

            xT = singles.tile([128, KO, N], mmdt)
            d_xT = nc.sync.dma_start(
                xT[:], xT_d.rearrange("(ko p) n -> p ko n", p=128)
            )
            wqk = singles.tile([128, KO, 2 * C], mmdt)
            d_wqk = nc.sync.dma_start(
                wqk[:], wqk_d.rearrange("(ko p) m -> p ko m", p=128)
            )
            wp = singles.tile([128, KO, C], mmdt)
            d_wp = nc.sync.dma_start(
                wp[:], wp_d.rearrange("(ko p) m -> p ko m", p=128)
            )
            bp = singles.tile([128, C], f32)
            d_bp = nc.sync.dma_start(
                bp[:],
                bass.AP(tensor=bp_d, offset=0, ap=[[0, 128], [1, C]]),
            )

            qt = singles.tile([128, HP, N], mmdt)
            kt = singles.tile([128, HP, NV], mmdt)
            vsb = singles.tile([128, NVT, H, HD + 1], mmdt)
            ou = singles.tile([128, HP, N], mmdt)      # unnormalized O^T (packed)
            rb = singles.tile([128, HP, N], mmdt)      # broadcast recips (packed)
            den = singles.tile([128, N], f32)          # denominators (mapped rows)
            rden = singles.tile([128, N], f32)
            rdb = singles.tile([128, N], mmdt)         # bf16 recip denominators
            rscratch = drampool.tile([H, N], mmdt)

            with (
                tc.tile_pool(name="qkv_psum", bufs=2, space="PSUM") as qp,
                tc.tile_pool(name="eb_pool", bufs=2) as eb_pool,
                tc.tile_pool(name="st_psum", bufs=2, space="PSUM") as st_psum,
                tc.tile_pool(name="pv_psum", bufs=len(ichunks), space="PSUM") as pv_psum,
                tc.tile_pool(name="e_pool", bufs=3) as e_pool,
                tc.tile_pool(name="p_pool", bufs=4) as p_pool,
                tc.tile_pool(name="drow_pool", bufs=3) as drow_pool,
            ):
                if HP >= 5:
                    # (first head, den partition start, n heads) by trigger pair
                    NORM_GROUPS = {
                        2: (0, 0, 6),
                        HP - 2: (6, 32, 2 * HP - 8),
                        HP - 1: (2 * HP - 2, 64, 2),
                    }
                else:
                    NORM_GROUPS = {HP - 1: (0, 0, H)}
                DEN_ROW = {}
                for _g0, _r0, _ng in NORM_GROUPS.values():
                    for _h in range(_g0, _g0 + _ng):
                        DEN_ROW[_h] = _r0 + _h - _g0

                def _normalize_group(g0, r0, ng):
                    nc.scalar.activation(
                        rden[r0 : r0 + ng], den[r0 : r0 + ng], Ln
                    )
                    nc.scalar.activation(
                        rdb[r0 : r0 + ng], rden[r0 : r0 + ng], Exp, scale=-1.0
                    )
                    nc.sync.dma_start(rscratch[g0 : g0 + ng], rdb[r0 : r0 + ng])
                    for h in range(g0, g0 + ng):
                        ho = 64 * (h % 2)
                        nc.sync.dma_start(
                            rb[ho : ho + 64, h // 2, :],
                            bass.AP(
                                tensor=rscratch.tensor,
                                offset=rscratch[h, 0].offset,
                                ap=[[0, 64], [1, N]],
                            ),
                        )
                    for sl in range(g0 // 2, (g0 + ng) // 2):
                        nc.vector.tensor_tensor(
                            ou[:, sl, :], ou[:, sl, :], rb[:, sl, :], mult
                        )

                # ---- V projection over compacted keys, masked, into [V|m] ----
                from concourse.tile import add_dep_helper
                _dma_anchor = None
                for jt in range(NVT):
                    for f0, fl in fchunks:
                        ps = qp.tile([128, 512], f32, tag="ps")
                        for ko in range(KO):
                            mm = nc.tensor.matmul(
                                ps[:, :fl],
                                lhsT=xkTs(ko)[:, 128 * jt : 128 * jt + 128],
                                rhs=wv[:, ko, f0 : f0 + fl],
                                start=(ko == 0),
                                stop=(ko == KO - 1),
                            )
                        if _dma_anchor is None:
                            _dma_anchor = mm.ins
                            for _d in (d_xT, d_wqk, d_wp, d_bp):
                                add_dep_helper(
                                    _d.ins,
                                    _dma_anchor,
                                    sync=True,
                                    reason="delay bulk loads behind urgent ones",
                                )
                        h0, nh = f0 // HD, fl // HD
                        nc.vector.tensor_tensor(
                            vsb[:, jt, h0 : h0 + nh, 0:HD],
                            ps[:, :fl].rearrange("p (h d) -> p h d", d=HD),
                            maskc[:, jt : jt + 1, None].to_broadcast([128, nh, HD]),
                            mult,
                        )
                    # mask column (the "ones" column that accumulates denom)
                    nc.vector.tensor_scalar_mul(
                        vsb[:, jt, :, HD : HD + 1],
                        maskc[:, jt : jt + 1, None].to_broadcast([128, H, 1]),
                        1.0,
                    )

                for pair in range(HP):
                    # Q^T (from x) / K^T (from compacted xk) for this pair
                    for mt in (pair, HP + pair):
                        if mt < HP:
                            dst, chunks = qt, ichunks
                        else:
                            dst, chunks = kt, kchunks
                        for i0, il in chunks:
                            ps = qp.tile([128, 512], f32, tag="ps")
                            for ko in range(KO):
                                rhs = (
                                    xT[:, ko, i0 : i0 + il]
                                    if mt < HP
                                    else xkTs(ko)[:, i0 : i0 + il]
                                )
                                nc.tensor.matmul(
                                    ps[:, :il],
                                    lhsT=wqk[:, ko, 128 * mt : 128 * mt + 128],
                                    rhs=rhs,
                                    start=(ko == 0),
                                    stop=(ko == KO - 1),
                                )
                            nc.vector.tensor_copy(
                                dst[:, pair, i0 : i0 + il], ps[:, :il]
                            )
                    # attention for the pair's two heads
                    for h in (2 * pair, 2 * pair + 1):
                        hp, ho = pair, 64 * (h % 2)
                        eb = eb_pool.tile([128, NVT, N], mmdt, tag="eb")
                        d_eb = nc.sync.dma_start(
                            eb[:], ebias_d[h].rearrange("(jt p) n -> p jt n", p=128)
                        )
                        if h < 2:
                            add_dep_helper(
                                d_eb.ins,
                                _dma_anchor,
                                sync=True,
                                reason="delay early ebias behind urgent loads",
                            )
                        pvs = [
                            pv_psum.tile([128, 512], f32, tag="pv", name=f"pv_{h}_{ic}")
                            for ic in range(len(ichunks))
                        ]
                        ptiles = {}
                        for jt in range(NVT + 2):
                            if jt < NVT:
                                st = st_psum.tile([128, N], f32, tag="st")
                                for i0, il in ichunks:
                                    nc.tensor.matmul(
                                        st[:, i0 : i0 + il],
                                        lhsT=kt[ho : ho + 64, hp, 128 * jt : 128 * jt + 128],
                                        rhs=qt[ho : ho + 64, hp, i0 : i0 + il],
                                        start=True,
                                        stop=True,
                                    )
                                e = e_pool.tile([128, N], mmdt, tag="e")
                                nc.scalar.activation(e[:], st[:], Exp)
                                p = p_pool.tile([128, N], mmdt, tag="p")
                                nc.vector.tensor_tensor(
                                    p[:], e[:], eb[:, jt, :], mult
                                )
                                ptiles[jt] = p
                            if jt >= 2:
                                jd = jt - 2
                                pd = ptiles.pop(jd)
                                for ic, (i0, il) in enumerate(ichunks):
                                    nc.tensor.matmul(
                                        pvs[ic][: HD + 1, :il],
                                        lhsT=vsb[:, jd, h, :],
                                        rhs=pd[:, i0 : i0 + il],
                                        start=(jd == 0),
                                        stop=(jd == NVT - 1),
                                    )
                        for ic, (i0, il) in enumerate(ichunks):
                            nc.vector.tensor_copy(
                                ou[ho : ho + 64, hp, i0 : i0 + il],
                                pvs[ic][:HD, :il],
                            )
                            drow = drow_pool.tile(
                                [128, 512], f32, tag="drow", name=f"drow_{h}_{ic}"
                            )
                            nc.scalar.copy(
                                drow[64:65, :il], pvs[ic][HD : HD + 1, :il]
                            )
                            dr = DEN_ROW[h]
                            nc.sync.dma_start(
                                den[dr : dr + 1, i0 : i0 + il], drow[64:65, :il]
                            )
                    # normalize in groups (32-aligned partition starts for the
                    # recip ops); overlaps later pairs' compute
                    if pair in NORM_GROUPS:
                        _normalize_group(*NORM_GROUPS[pair])

            # ---------------- output projection ----------------
            # ko 0..KO-2 run eagerly (their O^T slots normalize early); each
            # group's final-ko matmul (the slot normalized in the very last
            # group) is lagged so the normalize tail hides under real work.
            with (
                tc.tile_pool(name="proj_psum", bufs=7, space="PSUM") as proj_psum,
                tc.tile_pool(name="o_pool", bufs=3) as o_pool,
            ):
                groups = [(it, f0, fl) for it in range(NT) for f0, fl in fchunks]
                LAG = min(5, len(groups) - 1) if KO > 1 else 0
                psums = {}
                ots = {}
                for g in range(len(groups) + LAG):
                    if g < len(groups):
                        it, f0, fl = groups[g]
                        ps = proj_psum.tile(
                            [128, 512], f32, tag="ps", name=f"pps_{g}"
                        )
                        for ko in range(KO - 1):
                            nc.tensor.matmul(
                                ps[:, :fl],
                                lhsT=ou[:, ko, 128 * it : 128 * it + 128],
                                rhs=wp[:, ko, f0 : f0 + fl],
                                start=(ko == 0),
                                stop=False,
                            )
                        psums[g] = ps
                    if g >= LAG:
                        it, f0, fl = groups[g - LAG]
                        ps = psums.pop(g - LAG)
                        nc.tensor.matmul(
                            ps[:, :fl],
                            lhsT=ou[:, KO - 1, 128 * it : 128 * it + 128],
                            rhs=wp[:, KO - 1, f0 : f0 + fl],
                            start=(KO == 1),
                            stop=True,
                        )
                        if it not in ots:
                            ots[it] = o_pool.tile(
                                [128, C], f32, tag="ot", name=f"ot_{it}"
                            )
                        nc.vector.tensor_tensor(
                            ots[it][:, f0 : f0 + fl],
                            ps[:, :fl],
                            bp[:, f0 : f0 + fl],
                            add,
                        )
                        if f0 + fl >= C:
                            nc.sync.dma_start(
                                out_d[128 * it : 128 * it + 128, :], ots.pop(it)[:]
                            )

    nc.finalize()
    return nc


def _host_pack(x, w_qkv, w_proj, b_proj, bias_table, key_padding_mask,
               N=N_TOK, H=NUM_HEADS, mmdt_name="bfloat16"):
    """Host-side layout: per-core input dicts (core i <- batch i).
    Returns (in_maps, NVT)."""
    np_mmdt = ml_dtypes.bfloat16 if mmdt_name == "bfloat16" else np.float32
    C = H * HD

    x = np.asarray(x, np.float32)
    mask = np.asarray(key_padding_mask).astype(bool)
    Bb = x.shape[0]

    valid = [np.where(mask[b])[0] for b in range(Bb)]
    nv_max = max(1, max(len(v) for v in valid))
    NVT = (nv_max + 127) // 128
    NV = NVT * 128

    w_qkv = np.asarray(w_qkv, np.float32)
    wqk = np.ascontiguousarray(w_qkv[: 2 * C].T).astype(np.float32)
    wqk[:, :C] *= SCALE                       # fold softmax scale into W_q
    wqk = wqk.astype(np_mmdt)
    wv = np.ascontiguousarray(w_qkv[2 * C :].T).astype(np_mmdt)
    wp = np.ascontiguousarray(np.asarray(w_proj, np.float32).T).astype(np_mmdt)
    bp = np.asarray(b_proj, np.float32)

    etab = np.exp(np.asarray(bias_table, np.float32))   # [2N-1, H]
    iota = np.arange(N)

    in_maps = []
    for b in range(Bb):
        v = valid[b]
        nv = len(v)
        xT = np.ascontiguousarray(x[b].T).astype(np_mmdt)
        xk = np.zeros((NV, C), np.float32)
        xk[:nv] = x[b][v]
        xkT = np.ascontiguousarray(xk.T).astype(np_mmdt)
        mc = np.zeros(NV, np.float32)
        mc[:nv] = 1.0
        maskc = np.ascontiguousarray(mc.reshape(NVT, 128).T).astype(np_mmdt)
        # ebias[h, r, i] = exp(bias_table[v[r] - i + N - 1, h])
        idx = np.zeros((NV, N), np.int32)
        idx[:nv] = v[:, None] - iota[None, :] + N - 1
        eb = etab[idx, :]                     # [NV, N, H]
        eb[nv:] = 0.0
        ebias = np.ascontiguousarray(eb.transpose(2, 0, 1)).astype(np_mmdt)
        in_maps.append({
            "xT": xT, "xkT": xkT, "wqk": wqk, "wv": wv, "wp": wp,
            "ebias": ebias, "maskc": maskc, "bproj": bp,
        })
    return in_maps, NVT


def _run(x, w_qkv, w_proj, b_proj, bias_table, key_padding_mask, trace=False):
    from concourse.bass_utils import run_bass_kernel_spmd

    in_maps, NVT = _host_pack(
        x, w_qkv, w_proj, b_proj, bias_table, key_padding_mask
    )
    key = ("full", N_TOK, NUM_HEADS, NVT)
    if key not in _BUILD_CACHE:
        _BUILD_CACHE[key] = _build_nc(NVT=NVT)
    nc = _BUILD_CACHE[key]
    res = run_bass_kernel_spmd(nc, in_maps, core_ids=list(range(B)), trace=trace)
    out = np.stack([np.asarray(res.results[i]["out"]) for i in range(B)])
    return out.astype(np.float32), res


def kernel(x, w_qkv, w_proj, b_proj, bias_table, key_padding_mask):
    out, _ = _run(x, w_qkv, w_proj, b_proj, bias_table, key_padding_mask)
    return out
